# revision 47
# baseline (speedup 1.0000x reference)
"""Trainium2 Bass kernel for nn_BidirectionalTemporalAttention (v2).

Reference computation (B=2, T=16, F=128, D=1024, N=T*F=2048):
  xf = x.reshape(B, N, D)
  lookback branch: 8 heads, E=64, causal mask (keep k <= q)
  lookahead branch: 8 heads, anti-causal (keep k >= q)
  o = concat([o_lb, o_la], heads) -> (B, 16, N, 64) -> RAW reshape (B, N, D)
  out = o @ Wo^T -> (B, T, F, D)

The raw reshape means out row r = h*128 + g depends only on head h (tokens
16g..16g+15 of that head).  So with 4 heads per core each core's 512 output
rows are fully local: no collectives, the host just concatenates row slices.

Sharding over 8 cores: (batch b in 2) x (group in [lb0-3, lb4-7, la0-3, la4-7]).
Lookahead cores receive the token-reversed sequence so one SPMD causal program
serves all cores; their outputs are un-reversed on the host (row reversal
within each 128-row head block, plus a j-group reversal folded into Wo).

v2 performance design (vs the 240us fp32r v1):
  - fp16 operands everywhere on the PE (x, Wq/Wk/Wv, Q^T, K^T, Wo, O^T):
    LDWEIGHTS for 16-bit stationaries runs 4x faster than fp32r (the v1
    profile showed 132us of LDWEIGHTS), and matmuls keep 1 cycle/row at any
    moving size (fp32r drops to 4 cyc/row under 256).
  - P and V are bf16, not fp16: scores*scale span [-30, +17] for these
    inputs, so exp(s*scale - 4) needs ~e^30 of dynamic range - beyond fp16's
    ~1e12 but trivial for bf16's fp32-like exponent.  exp carries a -4.0 bias
    (cancels in the softmax ratio) to keep P comfortably inside bf16/fp32.
  - causal masking via the PE instead of the DVE: a [128,128] lower-triangle
    of -30000 is accumulated into the diagonal S^T blocks with an
    identity-stationary matmul before the exp; the DVE mask multiplies (40us
    in v1) disappear and the S->exp->PV chain never touches the DVE.
  - column-skipping: diagonal-block columns left of the triangle are fully
    masked, so S/exp/PV only cover [128*dg, 512) of each diagonal block.
  - softmax denominator from a bf16 ones-column appended to V (row 64 of the
    PV PSUM); 1/den via reciprocal_approx_fast (5x cheaper than the exact
    reciprocal, ~18 good bits); PSUM bounces ride the idle Pool engine.
  - software-pipelined issue order: S^T(kb+1) is issued before PV(kb) so the
    PE never sits behind the ACT exp; per chunk the issue order is
    attn(pair0) / project(chunk+1) / attn(pair1) so projection matmuls fill
    the PE while the ACT engine chews the previous attention's exps.
"""

import sys

if "/opt/trn_rl_repo" not in sys.path:
    sys.path.insert(0, "/opt/trn_rl_repo")

import numpy as np

import concourse.bass as bass  # noqa: F401
import concourse.mybir as mybir
import concourse.tile as tile
from concourse import bacc
from concourse import bass_utils as _bu
from concourse.bass_utils import run_bass_kernel_spmd

# walrus's ldw optimization (background weight-buffer loads) rejects the
# strided out-projection stationaries ("InstLdweights is not compatible with
# LDW optimization"), and was only worth ~4us on the fp32r kernel anyway.
_ENABLE_LDW_OPT = False
_orig_run_command = _bu.run_command


def _patched_run_command(cmd, *a, **kw):
    if _ENABLE_LDW_OPT and isinstance(cmd, list):
        cmd = [
            "--enable-ldw-opt=true" if c == "--enable-ldw-opt=false" else c
            for c in cmd
        ]
    return _orig_run_command(cmd, *a, **kw)


_bu.run_command = _patched_run_command

F32 = mybir.dt.float32
F16 = mybir.dt.float16
BF16 = mybir.dt.bfloat16
EXP = mybir.ActivationFunctionType.Exp
ACOPY = mybir.ActivationFunctionType.Copy

N = 2048  # tokens per batch
D = 1024  # embed dim
E = 64  # head dim
HPC = 4  # heads per core
NQB = 4  # query blocks of 512
NKB = 16  # key blocks of 128
NDB = 8  # d blocks of 128
SCALE = 0.125  # 1/sqrt(E)
CBIAS = 4.0  # exp(s*SCALE - CBIAS): cancels in softmax, keeps P in range
MASKNEG = -30000.0  # triangle bias; scale*(s+MASKNEG) << -90 -> exp == 0

_CACHE = {}


def build_nc():
    nc = bacc.Bacc("TRN2", target_bir_lowering=False, debug=False)

    xt_d = nc.dram_tensor("xt", [D, N], F16, kind="ExternalInput").ap()
    wq_d = nc.dram_tensor("wq", [128, NDB, 256], F16, kind="ExternalInput").ap()
    wk_d = nc.dram_tensor("wk", [128, NDB, 256], F16, kind="ExternalInput").ap()
    wv_d = nc.dram_tensor("wv", [128, NDB, 256], F16, kind="ExternalInput").ap()
    wo_d = nc.dram_tensor("wo", [128, 8, D], F16, kind="ExternalInput").ap()
    tri_d = nc.dram_tensor("tri", [128, 128], F16, kind="ExternalInput").ap()
    idn_d = nc.dram_tensor("idn", [128, 128], F16, kind="ExternalInput").ap()
    out_d = nc.dram_tensor("out", [512, D], F16, kind="ExternalOutput").ap()

    with tile.TileContext(nc) as tc:
        with (
            tc.tile_pool(name="w", bufs=1) as wp,
            tc.tile_pool(name="xp", bufs=3) as xp,
            tc.tile_pool(name="qkv", bufs=1) as qkvp,
            tc.tile_pool(name="pt", bufs=4) as ptp,
            tc.tile_pool(name="o2t", bufs=4) as o2tp,
            tc.tile_pool(name="ob", bufs=4) as obp,
            tc.tile_pool(name="rc", bufs=2) as rcp,
            tc.tile_pool(name="rc1", bufs=1) as rcp1,
            tc.tile_pool(name="pss", bufs=2, space="PSUM") as pss,
            tc.tile_pool(name="pso", bufs=2, space="PSUM") as pso,
            tc.tile_pool(name="psq", bufs=2, space="PSUM") as psq,
        ):
            # --- weights / constants (wq + x first: they gate the first MMs) ---
            wq_sb = wp.tile([128, NDB, 256], F16, tag="wq")
            wk_sb = wp.tile([128, NDB, 256], F16, tag="wk")
            wv_sb = wp.tile([128, NDB, 256], F16, tag="wv")
            tri_sb = wp.tile([128, 128], F16, tag="tri")
            idn_sb = wp.tile([128, 128], F16, tag="idn")
            xt_r = xt_d.rearrange("(o p) n -> p o n", p=128)
            # startup DMAs split across BOTH hardware DGE queues (sync + the
            # scalar engine, idle until the first exp); xc0 lands in do-pair
            # quarters so the first projections can start on partial data
            nc.sync.dma_start(wq_sb[:], wq_d)
            xc0 = xp.tile([128, NDB, 512], F16, tag="xc", name="xc_0")
            nc.scalar.dma_start(wk_sb[:], wk_d)
            for p in range(4):
                eng = nc.sync if p % 2 == 0 else nc.scalar
                eng.dma_start(
                    xc0[:, 2 * p : 2 * p + 2, :], xt_r[:, 2 * p : 2 * p + 2, 0:512]
                )
            nc.scalar.dma_start(wv_sb[:], wv_d)
            nc.sync.dma_start(tri_sb[:], tri_d)
            nc.scalar.dma_start(idn_sb[:], idn_d)
            # wo is only needed by the tail out-projection: its DMAs are
            # issued mid-loop (after the xc prefetches) to keep the input
            # path clear early on
            wo_parts = [
                wp.tile([128, 2, D], F16, tag=tg, name=f"wo_{i}")
                for i, tg in ((0, "wo2a"), (1, "wo2b"), (2, "wo2c"), (3, "wo2d"))
            ]
            ebias = wp.tile([128, 1], F32, tag="ebias")
            nc.vector.memset(ebias[:], -CBIAS)

            # persistent Q^T / K^T / V(+ones column)
            qt = qkvp.tile([128, 2, N], F16, tag="qt")  # [(2h)*64e, pair, n]
            kt = qkvp.tile([128, 2, N], F16, tag="kt")
            vt = qkvp.tile([128, NKB, HPC, 65], BF16, tag="vt")  # [k, kb, h, e|1]
            # DVE memset: keep the Pool engine's instruction stream pure
            # partition_broadcast (one gpsimd lib load, no drain thrash)
            nc.vector.memset(vt[:, :, :, 64], 1.0)

            o2t_all = {
                pr: [
                    o2tp.tile([128, N // 2], F16, tag="o2", name=f"o2t_{pr}_{i}")
                    for i in range(2)
                ]
                for pr in range(2)
            }
            pending = [None]

            def normalize_rest(pr, qb, osb_h, den2):
                # custom-DVE recip needs a partition-0 source: den rows are
                # gathered into den2 at bounce time
                rec = rcp1.tile([1, 2, 512], F32, tag="rec", name=f"rec_{pr}_{qb}")
                nc.vector.reciprocal_approx_fast(rec[:], den2[:])
                brs = []
                for h in range(2):
                    br = rcp1.tile(
                        [64, 512], F32, tag=f"brs{h}", name=f"brs_{pr}_{qb}_{h}"
                    )
                    nc.gpsimd.partition_broadcast(br[:], rec[0:1, h, :])
                    brs.append(br)
                for h in range(2):
                    for par in range(2):
                        # DVE, not Pool: Pool's strided tensor_tensor is slow
                        # (~835ns) and its lib-switch drains serialized the
                        # whole normalize tail behind the out-projection
                        nc.vector.tensor_mul(
                            o2t_all[pr][h][
                                64 * par : 64 * par + 64,
                                256 * qb : 256 * (qb + 1),
                            ],
                            osb_h[h][0:64, par::2],
                            brs[h][:, par::2],
                        )

            def attention(pr, qb):
                # run the deferred normalize first: its DVE/Pool work
                # overlaps this call's matmuls and completes the previous
                # pair's o2t early for the tail out-projection
                if pending[0] is not None:
                    normalize_rest(*pending[0])
                    pending[0] = None
                nkb = 4 * qb + 4  # kept key blocks (causal)
                o_ps = [
                    pso.tile([128, 512], F32, tag="ov", name=f"ov_{pr}_{qb}_{i}")
                    for i in range(2)
                ]
                pts = {}

                def s_block(kb):
                    dg = kb - 4 * qb  # diagonal mask pattern (0..3) if >= 0
                    qd = psq.tile(
                        [128, 2, 512], F32, tag="qd", name=f"qd_{pr}_{qb}_{kb}"
                    )
                    if dg >= 0:
                        c0 = 128 * dg
                        # PSUM start/stop is per 2KB bank (one bank per head
                        # here): exactly one start (the bias, which lazily
                        # zeroes the whole bank) and one stop (last S write).
                        for h in range(2):
                            nc.tensor.matmul(
                                qd[:, h, c0 : c0 + 128],
                                idn_sb[:],
                                tri_sb[:],
                                start=True,
                                stop=False,
                            )
                        for h in range(2):
                            kts = kt[64 * h : 64 * (h + 1), pr, kb * 128 : (kb + 1) * 128]
                            nc.tensor.matmul(
                                qd[:, h, c0 : c0 + 128],
                                kts,
                                qt[64 * h : 64 * (h + 1), pr,
                                   qb * 512 + c0 : qb * 512 + c0 + 128],
                                start=False,
                                stop=(dg == 3),
                            )
                            if dg < 3:
                                nc.tensor.matmul(
                                    qd[:, h, c0 + 128 : 512],
                                    kts,
                                    qt[64 * h : 64 * (h + 1), pr,
                                       qb * 512 + c0 + 128 : (qb + 1) * 512],
                                    start=False,
                                    stop=True,
                                )
                    else:
                        c0 = 0
                        for h in range(2):
                            nc.tensor.matmul(
                                qd[:, h, :],
                                kt[64 * h : 64 * (h + 1), pr, kb * 128 : (kb + 1) * 128],
                                qt[64 * h : 64 * (h + 1), pr, qb * 512 : (qb + 1) * 512],
                                start=True,
                                stop=True,
                            )
                    pt_t = ptp.tile(
                        [128, 2, 512], BF16, tag="pt", name=f"pt_{pr}_{qb}_{kb}"
                    )
                    nc.scalar.activation(
                        pt_t[:, :, c0:], qd[:, :, c0:], EXP,
                        scale=SCALE, bias=ebias[:],
                    )
                    pts[kb] = (pt_t, c0)

                def pv_block(kb):
                    dg = kb - 4 * qb
                    pt_t, c0 = pts.pop(kb)
                    for h in range(2):
                        v = vt[:, kb, 2 * pr + h, :]
                        if dg >= 0:
                            # one start per bank (kb==0 first piece; zeroes
                            # the bank lazily), one stop (dg==3 is last)
                            nc.tensor.matmul(
                                o_ps[h][0:65, c0 : c0 + 128],
                                v,
                                pt_t[:, h, c0 : c0 + 128],
                                start=(kb == 0),
                                stop=(dg == 3),
                            )
                            if dg < 3:
                                nc.tensor.matmul(
                                    o_ps[h][0:65, c0 + 128 : 512],
                                    v,
                                    pt_t[:, h, c0 + 128 : 512],
                                    start=False,
                                    stop=False,
                                )
                        else:
                            nc.tensor.matmul(
                                o_ps[h][0:65, :],
                                v,
                                pt_t[:, h, :],
                                start=(kb == 0),
                                stop=False,
                            )

                # software pipeline: S one key-block ahead of PV
                s_block(0)
                for kb in range(nkb):
                    if kb + 1 < nkb:
                        s_block(kb + 1)
                    pv_block(kb)

                # Bounce O~ + denominator row to SBUF; gpsimd cannot touch
                # PSUM, so split the copies between the DVE and the Scalar
                # engine (activation-Copy) — except in the last chunk, where
                # the exp stream is the critical path and ACT must stay clear.
                # The recip/broadcast/mult tail is deferred one call.
                osb_h = []
                den2 = rcp.tile([1, 2, 512], F32, tag="den2", name=f"den2_{pr}_{qb}")
                act_spare = qb < 3 or pr == 1  # after the last exp ACT is free
                for h in range(2):
                    osb = o2tp.tile(
                        [65, 512], F32, tag="osb", name=f"osb_{pr}_{qb}_{h}"
                    )
                    if act_spare and h == 0:
                        nc.scalar.activation(osb[:], o_ps[h][0:65, :], ACOPY)
                    else:
                        nc.vector.tensor_copy(osb[:], o_ps[h][0:65, :])
                    osb_h.append(osb)
                    if act_spare:
                        nc.scalar.activation(
                            den2[0:1, h, :], o_ps[h][64:65, :], ACOPY
                        )
                    else:
                        nc.vector.tensor_copy(den2[0:1, h, :], o_ps[h][64:65, :])
                pending[0] = (pr, qb, osb_h, den2)

            def project(c, xc):
                if c == 0:
                    # startup: x arrives in do-pair quarters; keep all 4 QK
                    # chains open (Q pair in pss, K pair in a borrowed qd
                    # tile's half-banks) and feed each quarter as it lands,
                    # so the PE works while the rest of x streams in
                    qps = [pss.tile([128, 512], F32, tag="sc", name=f"qp0_{mg}")
                           for mg in range(2)]
                    kps = psq.tile([128, 2, 512], F32, tag="qd", name="kp0")
                    for p in range(4):
                        for mg in range(2):
                            for do in (2 * p, 2 * p + 1):
                                nc.tensor.matmul(
                                    qps[mg][:],
                                    wq_sb[:, do, mg * 128 : (mg + 1) * 128],
                                    xc[:, do, :],
                                    start=(do == 0),
                                    stop=(do == NDB - 1),
                                )
                        for mg in range(2):
                            for do in (2 * p, 2 * p + 1):
                                nc.tensor.matmul(
                                    kps[:, mg, :],
                                    wk_sb[:, do, mg * 128 : (mg + 1) * 128],
                                    xc[:, do, :],
                                    start=(do == 0),
                                    stop=(do == NDB - 1),
                                )
                    for mg in range(2):
                        nc.vector.tensor_copy(qt[:, mg, 0:512], qps[mg][:])
                        nc.vector.tensor_copy(kt[:, mg, 0:512], kps[:, mg, :])
                else:
                    for w_sb, dst in ((wq_sb, qt), (wk_sb, kt)):
                        for mg in range(2):
                            ps = pss.tile([128, 512], F32, tag="sc")
                            for do in range(NDB):
                                nc.tensor.matmul(
                                    ps[:],
                                    w_sb[:, do, mg * 128 : (mg + 1) * 128],
                                    xc[:, do, :],
                                    start=(do == 0),
                                    stop=(do == NDB - 1),
                                )
                            nc.vector.tensor_copy(
                                dst[:, mg, c * 512 : (c + 1) * 512], ps[:]
                            )
                for kbl in range(4):
                    kb = 4 * c + kbl
                    ps = pss.tile([128, 512], F32, tag="sc")
                    nps = ps[:, 0:256]
                    for do in range(NDB):
                        nc.tensor.matmul(
                            nps,
                            xc[:, do, kbl * 128 : (kbl + 1) * 128],
                            wv_sb[:, do, :],
                            start=(do == 0),
                            stop=(do == NDB - 1),
                        )
                    nc.vector.tensor_copy(
                        vt[:, kb, :, 0:64], nps.rearrange("p (h e) -> p h e", h=HPC)
                    )

            # --- fused pipeline: attn(pair0, qb=c) | project chunk c+1 |
            #     attn(pair1, qb=c) - projection matmuls fill the PE while
            #     the ACT engine runs pair0's exps ---
            def out_proj_steps(pr):
                # out rows for head hl = 2*pr + h
                for h in range(2):
                    hl = 2 * pr + h
                    for oh in range(2):
                        op = pss.tile([128, 512], F32, tag="sc", name=f"op_{hl}_{oh}")
                        for m in range(8):
                            nc.tensor.matmul(
                                op[:],
                                o2t_all[pr][h][:, m::8],
                                wo_parts[m // 2][:, m % 2, oh * 512 : (oh + 1) * 512],
                                start=(m == 0),
                                stop=(m == 7),
                            )
                            if m % 4 == 3:
                                yield
                        osb = obp.tile([128, 512], F16, tag="ob", name=f"ob_{hl}_{oh}")
                        # tail: both DVE and ACT are idle; alternate the
                        # PSUM bounces so the last chains drain in parallel
                        if oh == 0:
                            nc.scalar.activation(osb[:], op[:], ACOPY)
                            dma_eng = nc.scalar
                        else:
                            nc.vector.tensor_copy(osb[:], op[:])
                            dma_eng = nc.sync
                        dma_eng.dma_start(
                            out_d[hl * 128 : (hl + 1) * 128, oh * 512 : (oh + 1) * 512],
                            osb[:],
                        )
                        yield

            xcs = {0: xc0}
            for c in range(1, NQB):
                xc = xp.tile([128, NDB, 512], F16, tag="xc", name=f"xc_{c}")
                xcs[c] = xc
            project(0, xc0)
            for c in range(NQB):
                if c + 1 < NQB:  # prefetch next chunk before the attention
                    xc = xcs[c + 1]
                    nc.sync.dma_start(
                        xc[:, 0:4, :], xt_r[:, 0:4, (c + 1) * 512 : (c + 2) * 512]
                    )
                    nc.sync.dma_start(
                        xc[:, 4:8, :], xt_r[:, 4:8, (c + 1) * 512 : (c + 2) * 512]
                    )
                if c == 1:
                    for i in range(4):
                        nc.sync.dma_start(
                            wo_parts[i][:], wo_d[:, 2 * i : 2 * i + 2, :]
                        )
                attention(0, qb=c)
                if c + 1 < NQB:
                    project(c + 1, xcs[c + 1])
                attention(1, qb=c)
            # tail: flush the last normalize first so its DVE/Pool chain
            # runs under pair-0's out-projection matmuls
            if pending[0] is not None:
                normalize_rest(*pending[0])
                pending[0] = None
            for _ in out_proj_steps(0):
                pass
            for _ in out_proj_steps(1):
                pass

    nc.compile()
    return nc


def _get_nc():
    if "nc" not in _CACHE:
        _CACHE["nc"] = build_nc()
    return _CACHE["nc"]


def _prep_w(wg):
    """(4, 64, 1024) per-head weights -> [128, 8, 256] SBUF lhsT layout."""
    # WT[d, f=(h*64+e)] = wg[h, e, d]; block d = do*128 + p -> [p, do, f]
    wt = wg.transpose(2, 0, 1).reshape(D, 256)
    return np.ascontiguousarray(
        wt.reshape(NDB, 128, 256).transpose(1, 0, 2).astype(np.float16)
    )


def _prep_wo(wot):
    """WoT (1024, 1024) [c, o] -> [128, 8, 1024] with c = 128*m + p."""
    return np.ascontiguousarray(
        wot.reshape(8, 128, D).transpose(1, 0, 2).astype(np.float16)
    )


def make_in_maps(x, Wq_lb, Wk_lb, Wv_lb, Wq_la, Wk_la, Wv_la, Wo):
    B = x.shape[0]
    xf = np.asarray(x, np.float32).reshape(B, N, D)
    wot = np.ascontiguousarray(np.asarray(Wo, np.float32).T)  # [c, o]
    wot_rev = np.ascontiguousarray(wot.reshape(16, 64, D)[::-1].reshape(D, D))
    wo_maps = {False: _prep_wo(wot), True: _prep_wo(wot_rev)}

    kp = np.arange(128)[:, None]
    jj = np.arange(128)[None, :]
    tri = np.where(jj < kp, np.float16(MASKNEG), np.float16(0)).astype(np.float16)
    idn = np.eye(128, dtype=np.float16)

    xts = {}
    for b in range(B):
        xts[(b, False)] = np.ascontiguousarray(xf[b].T.astype(np.float16))
        xts[(b, True)] = np.ascontiguousarray(xf[b][::-1].T.astype(np.float16))

    wsel = {
        False: (np.asarray(Wq_lb, np.float32), np.asarray(Wk_lb, np.float32),
                np.asarray(Wv_lb, np.float32)),
        True: (np.asarray(Wq_la, np.float32), np.asarray(Wk_la, np.float32),
               np.asarray(Wv_la, np.float32)),
    }
    wcache = {}
    in_maps = []
    for c in range(8):
        b, grp = divmod(c, 4)
        la = grp >= 2
        half = grp % 2
        key = (la, half)
        if key not in wcache:
            wq, wk, wv = wsel[la]
            sl = slice(half * 4, half * 4 + 4)
            wcache[key] = (_prep_w(wq[sl]), _prep_w(wk[sl]), _prep_w(wv[sl]))
        pwq, pwk, pwv = wcache[key]
        in_maps.append(
            {
                "xt": xts[(b, la)],
                "wq": pwq,
                "wk": pwk,
                "wv": pwv,
                "wo": wo_maps[la],
                "tri": tri,
                "idn": idn,
            }
        )
    return in_maps


def assemble(results, B=2):
    out = np.empty((B, N, D), np.float32)
    for c in range(8):
        b, grp = divmod(c, 4)
        # device output is f16; upcast on host (kernel returns fp32 like x)
        part = np.asarray(results[c]["out"]).astype(np.float32)  # (512, 1024)
        if grp >= 2:  # lookahead: un-reverse rows within each head block
            part = part.reshape(HPC, 128, D)[:, ::-1].reshape(512, D)
        out[b, grp * 512 : (grp + 1) * 512] = part
    return out


def kernel(x, Wq_lb, Wk_lb, Wv_lb, Wq_la, Wk_la, Wv_la, Wo):
    nc = _get_nc()
    in_maps = make_in_maps(x, Wq_lb, Wk_lb, Wv_lb, Wq_la, Wk_la, Wv_la, Wo)
    res = run_bass_kernel_spmd(nc, in_maps, list(range(8)))
    B, T, F_, D_ = x.shape
    return assemble(res.results, B).reshape(B, T, F_, D_)


# revision 48
# speedup vs baseline: 1.1755x; 1.1755x over previous
"""Trainium2 Bass kernel for nn_BidirectionalTemporalAttention (v2).

Reference computation (B=2, T=16, F=128, D=1024, N=T*F=2048):
  xf = x.reshape(B, N, D)
  lookback branch: 8 heads, E=64, causal mask (keep k <= q)
  lookahead branch: 8 heads, anti-causal (keep k >= q)
  o = concat([o_lb, o_la], heads) -> (B, 16, N, 64) -> RAW reshape (B, N, D)
  out = o @ Wo^T -> (B, T, F, D)

The raw reshape means out row r = h*128 + g depends only on head h (tokens
16g..16g+15 of that head).  So with 4 heads per core each core's 512 output
rows are fully local: no collectives, the host just concatenates row slices.

Sharding over 8 cores: (batch b in 2) x (group in [lb0-3, lb4-7, la0-3, la4-7]).
Lookahead cores receive the token-reversed sequence so one SPMD causal program
serves all cores; their outputs are un-reversed on the host (row reversal
within each 128-row head block, plus a j-group reversal folded into Wo).

v2 performance design (vs the 240us fp32r v1):
  - fp16 operands everywhere on the PE (x, Wq/Wk/Wv, Q^T, K^T, Wo, O^T):
    LDWEIGHTS for 16-bit stationaries runs 4x faster than fp32r (the v1
    profile showed 132us of LDWEIGHTS), and matmuls keep 1 cycle/row at any
    moving size (fp32r drops to 4 cyc/row under 256).
  - P and V are bf16, not fp16: scores*scale span [-30, +17] for these
    inputs, so exp(s*scale - 4) needs ~e^30 of dynamic range - beyond fp16's
    ~1e12 but trivial for bf16's fp32-like exponent.  exp carries a -4.0 bias
    (cancels in the softmax ratio) to keep P comfortably inside bf16/fp32.
  - causal masking via the PE instead of the DVE: a [128,128] lower-triangle
    of -30000 is accumulated into the diagonal S^T blocks with an
    identity-stationary matmul before the exp; the DVE mask multiplies (40us
    in v1) disappear and the S->exp->PV chain never touches the DVE.
  - column-skipping: diagonal-block columns left of the triangle are fully
    masked, so S/exp/PV only cover [128*dg, 512) of each diagonal block.
  - softmax denominator from a bf16 ones-column appended to V (row 64 of the
    PV PSUM); 1/den via reciprocal_approx_fast (5x cheaper than the exact
    reciprocal, ~18 good bits); PSUM bounces ride the idle Pool engine.
  - software-pipelined issue order: S^T(kb+1) is issued before PV(kb) so the
    PE never sits behind the ACT exp; per chunk the issue order is
    attn(pair0) / project(chunk+1) / attn(pair1) so projection matmuls fill
    the PE while the ACT engine chews the previous attention's exps.
"""

import sys

if "/opt/trn_rl_repo" not in sys.path:
    sys.path.insert(0, "/opt/trn_rl_repo")

import numpy as np

import concourse.bass as bass  # noqa: F401
import concourse.mybir as mybir
import concourse.tile as tile
from concourse import bacc
from concourse import bass_utils as _bu
from concourse.bass_utils import run_bass_kernel_spmd

# walrus's ldw optimization (background weight-buffer loads) rejects the
# strided out-projection stationaries ("InstLdweights is not compatible with
# LDW optimization"), and was only worth ~4us on the fp32r kernel anyway.
_ENABLE_LDW_OPT = False
_orig_run_command = _bu.run_command


def _patched_run_command(cmd, *a, **kw):
    if _ENABLE_LDW_OPT and isinstance(cmd, list):
        cmd = [
            "--enable-ldw-opt=true" if c == "--enable-ldw-opt=false" else c
            for c in cmd
        ]
    return _orig_run_command(cmd, *a, **kw)


_bu.run_command = _patched_run_command

F32 = mybir.dt.float32
F16 = mybir.dt.float16
BF16 = mybir.dt.bfloat16
EXP = mybir.ActivationFunctionType.Exp
ACOPY = mybir.ActivationFunctionType.Copy

N = 2048  # tokens per batch
D = 1024  # embed dim
E = 64  # head dim
HPC = 4  # heads per core
NQB = 4  # query blocks of 512
NKB = 16  # key blocks of 128
NDB = 8  # d blocks of 128
SCALE = 0.125  # 1/sqrt(E)
CBIAS = 4.0  # exp(s*SCALE - CBIAS): cancels in softmax, keeps P in range
MASKNEG = -30000.0  # triangle bias; scale*(s+MASKNEG) << -90 -> exp == 0

_CACHE = {}


def build_nc():
    nc = bacc.Bacc("TRN2", target_bir_lowering=False, debug=False)

    xt_d = nc.dram_tensor("xt", [D, N], F16, kind="ExternalInput").ap()
    wq_d = nc.dram_tensor("wq", [128, NDB, 256], F16, kind="ExternalInput").ap()
    wk_d = nc.dram_tensor("wk", [128, NDB, 256], F16, kind="ExternalInput").ap()
    wv_d = nc.dram_tensor("wv", [128, NDB, 256], F16, kind="ExternalInput").ap()
    wo_d = nc.dram_tensor("wo", [128, 8, D], F16, kind="ExternalInput").ap()
    tri_d = nc.dram_tensor("tri", [128, 128], F16, kind="ExternalInput").ap()
    idn_d = nc.dram_tensor("idn", [128, 128], F16, kind="ExternalInput").ap()
    out_d = nc.dram_tensor("out", [512, D], F32, kind="ExternalOutput").ap()

    with tile.TileContext(nc) as tc:
        with (
            tc.tile_pool(name="w", bufs=1) as wp,
            tc.tile_pool(name="xp", bufs=3) as xp,
            tc.tile_pool(name="qkv", bufs=1) as qkvp,
            tc.tile_pool(name="pt", bufs=4) as ptp,
            tc.tile_pool(name="o2t", bufs=4) as o2tp,
            tc.tile_pool(name="ob", bufs=4) as obp,
            tc.tile_pool(name="rc", bufs=2) as rcp,
            tc.tile_pool(name="rc1", bufs=1) as rcp1,
            tc.tile_pool(name="pss", bufs=2, space="PSUM") as pss,
            tc.tile_pool(name="pso", bufs=2, space="PSUM") as pso,
            tc.tile_pool(name="psq", bufs=2, space="PSUM") as psq,
        ):
            # --- weights / constants (wq + x first: they gate the first MMs) ---
            wq_sb = wp.tile([128, NDB, 256], F16, tag="wq")
            wk_sb = wp.tile([128, NDB, 256], F16, tag="wk")
            wv_sb = wp.tile([128, NDB, 256], F16, tag="wv")
            tri_sb = wp.tile([128, 128], F16, tag="tri")
            idn_sb = wp.tile([128, 128], F16, tag="idn")
            xt_r = xt_d.rearrange("(o p) n -> p o n", p=128)
            # startup DMAs split across BOTH hardware DGE queues (sync + the
            # scalar engine, idle until the first exp); xc0 lands in do-pair
            # quarters so the first projections can start on partial data
            nc.sync.dma_start(wq_sb[:], wq_d)
            xc0 = xp.tile([128, NDB, 512], F16, tag="xc", name="xc_0")
            nc.scalar.dma_start(wk_sb[:], wk_d)
            for p in range(4):
                eng = nc.sync if p % 2 == 0 else nc.scalar
                eng.dma_start(
                    xc0[:, 2 * p : 2 * p + 2, :], xt_r[:, 2 * p : 2 * p + 2, 0:512]
                )
            nc.scalar.dma_start(wv_sb[:], wv_d)
            nc.sync.dma_start(tri_sb[:], tri_d)
            nc.scalar.dma_start(idn_sb[:], idn_d)
            # wo is only needed by the tail out-projection: its DMAs are
            # issued mid-loop (after the xc prefetches) to keep the input
            # path clear early on
            wo_parts = [
                wp.tile([128, 2, D], F16, tag=tg, name=f"wo_{i}")
                for i, tg in ((0, "wo2a"), (1, "wo2b"), (2, "wo2c"), (3, "wo2d"))
            ]
            ebias = wp.tile([128, 1], F32, tag="ebias")
            nc.vector.memset(ebias[:], -CBIAS)

            # persistent Q^T / K^T / V(+ones column)
            qt = qkvp.tile([128, 2, N], F16, tag="qt")  # [(2h)*64e, pair, n]
            kt = qkvp.tile([128, 2, N], F16, tag="kt")
            vt = qkvp.tile([128, NKB, HPC, 65], BF16, tag="vt")  # [k, kb, h, e|1]
            # DVE memset: keep the Pool engine's instruction stream pure
            # partition_broadcast (one gpsimd lib load, no drain thrash)
            nc.vector.memset(vt[:, :, :, 64], 1.0)

            o2t_all = {
                pr: [
                    o2tp.tile([128, N // 2], F16, tag="o2", name=f"o2t_{pr}_{i}")
                    for i in range(2)
                ]
                for pr in range(2)
            }
            pending = [None]

            def normalize_rest(pr, qb, osb_h, den2):
                # custom-DVE recip needs a partition-0 source: den rows are
                # gathered into den2 at bounce time
                rec = rcp1.tile([1, 2, 512], F32, tag="rec", name=f"rec_{pr}_{qb}")
                nc.vector.reciprocal_approx_fast(rec[:], den2[:])
                brs = []
                for h in range(2):
                    br = rcp1.tile(
                        [64, 512], F32, tag=f"brs{h}", name=f"brs_{pr}_{qb}_{h}"
                    )
                    nc.gpsimd.partition_broadcast(br[:], rec[0:1, h, :])
                    brs.append(br)
                for h in range(2):
                    for par in range(2):
                        # DVE, not Pool: Pool's strided tensor_tensor is slow
                        # (~835ns) and its lib-switch drains serialized the
                        # whole normalize tail behind the out-projection
                        nc.vector.tensor_mul(
                            o2t_all[pr][h][
                                64 * par : 64 * par + 64,
                                256 * qb : 256 * (qb + 1),
                            ],
                            osb_h[h][0:64, par::2],
                            brs[h][:, par::2],
                        )

            def attention(pr, qb):
                # run the deferred normalize first: its DVE/Pool work
                # overlaps this call's matmuls and completes the previous
                # pair's o2t early for the tail out-projection
                if pending[0] is not None:
                    normalize_rest(*pending[0])
                    pending[0] = None
                nkb = 4 * qb + 4  # kept key blocks (causal)
                o_ps = [
                    pso.tile([128, 512], F32, tag="ov", name=f"ov_{pr}_{qb}_{i}")
                    for i in range(2)
                ]
                pts = {}

                def s_block(kb):
                    dg = kb - 4 * qb  # diagonal mask pattern (0..3) if >= 0
                    qd = psq.tile(
                        [128, 2, 512], F32, tag="qd", name=f"qd_{pr}_{qb}_{kb}"
                    )
                    if dg >= 0:
                        c0 = 128 * dg
                        # PSUM start/stop is per 2KB bank (one bank per head
                        # here): exactly one start (the bias, which lazily
                        # zeroes the whole bank) and one stop (last S write).
                        for h in range(2):
                            nc.tensor.matmul(
                                qd[:, h, c0 : c0 + 128],
                                idn_sb[:],
                                tri_sb[:],
                                start=True,
                                stop=False,
                            )
                        for h in range(2):
                            kts = kt[64 * h : 64 * (h + 1), pr, kb * 128 : (kb + 1) * 128]
                            nc.tensor.matmul(
                                qd[:, h, c0 : c0 + 128],
                                kts,
                                qt[64 * h : 64 * (h + 1), pr,
                                   qb * 512 + c0 : qb * 512 + c0 + 128],
                                start=False,
                                stop=(dg == 3),
                            )
                            if dg < 3:
                                nc.tensor.matmul(
                                    qd[:, h, c0 + 128 : 512],
                                    kts,
                                    qt[64 * h : 64 * (h + 1), pr,
                                       qb * 512 + c0 + 128 : (qb + 1) * 512],
                                    start=False,
                                    stop=True,
                                )
                    else:
                        c0 = 0
                        for h in range(2):
                            nc.tensor.matmul(
                                qd[:, h, :],
                                kt[64 * h : 64 * (h + 1), pr, kb * 128 : (kb + 1) * 128],
                                qt[64 * h : 64 * (h + 1), pr, qb * 512 : (qb + 1) * 512],
                                start=True,
                                stop=True,
                            )
                    pt_t = ptp.tile(
                        [128, 2, 512], BF16, tag="pt", name=f"pt_{pr}_{qb}_{kb}"
                    )
                    nc.scalar.activation(
                        pt_t[:, :, c0:], qd[:, :, c0:], EXP,
                        scale=SCALE, bias=ebias[:],
                    )
                    pts[kb] = (pt_t, c0)

                def pv_block(kb):
                    dg = kb - 4 * qb
                    pt_t, c0 = pts.pop(kb)
                    for h in range(2):
                        v = vt[:, kb, 2 * pr + h, :]
                        if dg >= 0:
                            # one start per bank (kb==0 first piece; zeroes
                            # the bank lazily), one stop (dg==3 is last)
                            nc.tensor.matmul(
                                o_ps[h][0:65, c0 : c0 + 128],
                                v,
                                pt_t[:, h, c0 : c0 + 128],
                                start=(kb == 0),
                                stop=(dg == 3),
                            )
                            if dg < 3:
                                nc.tensor.matmul(
                                    o_ps[h][0:65, c0 + 128 : 512],
                                    v,
                                    pt_t[:, h, c0 + 128 : 512],
                                    start=False,
                                    stop=False,
                                )
                        else:
                            nc.tensor.matmul(
                                o_ps[h][0:65, :],
                                v,
                                pt_t[:, h, :],
                                start=(kb == 0),
                                stop=False,
                            )

                # software pipeline: S one key-block ahead of PV
                s_block(0)
                for kb in range(nkb):
                    if kb + 1 < nkb:
                        s_block(kb + 1)
                    pv_block(kb)

                # Bounce O~ + denominator row to SBUF; gpsimd cannot touch
                # PSUM, so split the copies between the DVE and the Scalar
                # engine (activation-Copy) — except in the last chunk, where
                # the exp stream is the critical path and ACT must stay clear.
                # The recip/broadcast/mult tail is deferred one call.
                osb_h = []
                den2 = rcp.tile([1, 2, 512], F32, tag="den2", name=f"den2_{pr}_{qb}")
                act_spare = qb < 3 or pr == 1  # after the last exp ACT is free
                for h in range(2):
                    osb = o2tp.tile(
                        [65, 512], F32, tag="osb", name=f"osb_{pr}_{qb}_{h}"
                    )
                    if act_spare and h == 0:
                        nc.scalar.activation(osb[:], o_ps[h][0:65, :], ACOPY)
                    else:
                        nc.vector.tensor_copy(osb[:], o_ps[h][0:65, :])
                    osb_h.append(osb)
                    if act_spare:
                        nc.scalar.activation(
                            den2[0:1, h, :], o_ps[h][64:65, :], ACOPY
                        )
                    else:
                        nc.vector.tensor_copy(den2[0:1, h, :], o_ps[h][64:65, :])
                pending[0] = (pr, qb, osb_h, den2)

            def project(c, xc):
                if c == 0:
                    # startup: x arrives in do-pair quarters; keep all 4 QK
                    # chains open (Q pair in pss, K pair in a borrowed qd
                    # tile's half-banks) and feed each quarter as it lands,
                    # so the PE works while the rest of x streams in
                    qps = [pss.tile([128, 512], F32, tag="sc", name=f"qp0_{mg}")
                           for mg in range(2)]
                    kps = psq.tile([128, 2, 512], F32, tag="qd", name="kp0")
                    for p in range(4):
                        for mg in range(2):
                            for do in (2 * p, 2 * p + 1):
                                nc.tensor.matmul(
                                    qps[mg][:],
                                    wq_sb[:, do, mg * 128 : (mg + 1) * 128],
                                    xc[:, do, :],
                                    start=(do == 0),
                                    stop=(do == NDB - 1),
                                )
                        for mg in range(2):
                            for do in (2 * p, 2 * p + 1):
                                nc.tensor.matmul(
                                    kps[:, mg, :],
                                    wk_sb[:, do, mg * 128 : (mg + 1) * 128],
                                    xc[:, do, :],
                                    start=(do == 0),
                                    stop=(do == NDB - 1),
                                )
                    for mg in range(2):
                        nc.vector.tensor_copy(qt[:, mg, 0:512], qps[mg][:])
                        nc.vector.tensor_copy(kt[:, mg, 0:512], kps[:, mg, :])
                else:
                    for w_sb, dst in ((wq_sb, qt), (wk_sb, kt)):
                        for mg in range(2):
                            ps = pss.tile([128, 512], F32, tag="sc")
                            for do in range(NDB):
                                nc.tensor.matmul(
                                    ps[:],
                                    w_sb[:, do, mg * 128 : (mg + 1) * 128],
                                    xc[:, do, :],
                                    start=(do == 0),
                                    stop=(do == NDB - 1),
                                )
                            nc.vector.tensor_copy(
                                dst[:, mg, c * 512 : (c + 1) * 512], ps[:]
                            )
                for kbl in range(4):
                    kb = 4 * c + kbl
                    ps = pss.tile([128, 512], F32, tag="sc")
                    nps = ps[:, 0:256]
                    for do in range(NDB):
                        nc.tensor.matmul(
                            nps,
                            xc[:, do, kbl * 128 : (kbl + 1) * 128],
                            wv_sb[:, do, :],
                            start=(do == 0),
                            stop=(do == NDB - 1),
                        )
                    nc.vector.tensor_copy(
                        vt[:, kb, :, 0:64], nps.rearrange("p (h e) -> p h e", h=HPC)
                    )

            # --- fused pipeline: attn(pair0, qb=c) | project chunk c+1 |
            #     attn(pair1, qb=c) - projection matmuls fill the PE while
            #     the ACT engine runs pair0's exps ---
            def out_proj_steps(pr):
                # out rows for head hl = 2*pr + h
                for h in range(2):
                    hl = 2 * pr + h
                    for oh in range(2):
                        op = pss.tile([128, 512], F32, tag="sc", name=f"op_{hl}_{oh}")
                        for m in range(8):
                            nc.tensor.matmul(
                                op[:],
                                o2t_all[pr][h][:, m::8],
                                wo_parts[m // 2][:, m % 2, oh * 512 : (oh + 1) * 512],
                                start=(m == 0),
                                stop=(m == 7),
                            )
                            if m % 4 == 3:
                                yield
                        osb = obp.tile([128, 512], F32, tag="ob", name=f"ob_{hl}_{oh}")
                        # tail: both DVE and ACT are idle; alternate the
                        # PSUM bounces so the last chains drain in parallel
                        if oh == 0:
                            nc.scalar.activation(osb[:], op[:], ACOPY)
                            dma_eng = nc.scalar
                        else:
                            nc.vector.tensor_copy(osb[:], op[:])
                            dma_eng = nc.sync
                        dma_eng.dma_start(
                            out_d[hl * 128 : (hl + 1) * 128, oh * 512 : (oh + 1) * 512],
                            osb[:],
                        )
                        yield

            xcs = {0: xc0}
            for c in range(1, NQB):
                xc = xp.tile([128, NDB, 512], F16, tag="xc", name=f"xc_{c}")
                xcs[c] = xc
            project(0, xc0)
            for c in range(NQB):
                if c + 1 < NQB:  # prefetch next chunk before the attention
                    xc = xcs[c + 1]
                    nc.sync.dma_start(
                        xc[:, 0:4, :], xt_r[:, 0:4, (c + 1) * 512 : (c + 2) * 512]
                    )
                    nc.sync.dma_start(
                        xc[:, 4:8, :], xt_r[:, 4:8, (c + 1) * 512 : (c + 2) * 512]
                    )
                if c == 1:
                    for i in range(4):
                        nc.sync.dma_start(
                            wo_parts[i][:], wo_d[:, 2 * i : 2 * i + 2, :]
                        )
                attention(0, qb=c)
                if c + 1 < NQB:
                    project(c + 1, xcs[c + 1])
                attention(1, qb=c)
            # tail: flush the last normalize first so its DVE/Pool chain
            # runs under pair-0's out-projection matmuls
            if pending[0] is not None:
                normalize_rest(*pending[0])
                pending[0] = None
            for _ in out_proj_steps(0):
                pass
            for _ in out_proj_steps(1):
                pass

    nc.compile()
    return nc


def _get_nc():
    if "nc" not in _CACHE:
        _CACHE["nc"] = build_nc()
    return _CACHE["nc"]


def _prep_w(wg):
    """(4, 64, 1024) per-head weights -> [128, 8, 256] SBUF lhsT layout."""
    # WT[d, f=(h*64+e)] = wg[h, e, d]; block d = do*128 + p -> [p, do, f]
    wt = wg.transpose(2, 0, 1).reshape(D, 256)
    return np.ascontiguousarray(
        wt.reshape(NDB, 128, 256).transpose(1, 0, 2).astype(np.float16)
    )


def _prep_wo(wot):
    """WoT (1024, 1024) [c, o] -> [128, 8, 1024] with c = 128*m + p."""
    return np.ascontiguousarray(
        wot.reshape(8, 128, D).transpose(1, 0, 2).astype(np.float16)
    )


def make_in_maps(x, Wq_lb, Wk_lb, Wv_lb, Wq_la, Wk_la, Wv_la, Wo):
    B = x.shape[0]
    xf = np.asarray(x, np.float32).reshape(B, N, D)
    wot = np.ascontiguousarray(np.asarray(Wo, np.float32).T)  # [c, o]
    wot_rev = np.ascontiguousarray(wot.reshape(16, 64, D)[::-1].reshape(D, D))
    wo_maps = {False: _prep_wo(wot), True: _prep_wo(wot_rev)}

    kp = np.arange(128)[:, None]
    jj = np.arange(128)[None, :]
    tri = np.where(jj < kp, np.float16(MASKNEG), np.float16(0)).astype(np.float16)
    idn = np.eye(128, dtype=np.float16)

    xts = {}
    for b in range(B):
        xts[(b, False)] = np.ascontiguousarray(xf[b].T.astype(np.float16))
        xts[(b, True)] = np.ascontiguousarray(xf[b][::-1].T.astype(np.float16))

    wsel = {
        False: (np.asarray(Wq_lb, np.float32), np.asarray(Wk_lb, np.float32),
                np.asarray(Wv_lb, np.float32)),
        True: (np.asarray(Wq_la, np.float32), np.asarray(Wk_la, np.float32),
               np.asarray(Wv_la, np.float32)),
    }
    wcache = {}
    in_maps = []
    for c in range(8):
        b, grp = divmod(c, 4)
        la = grp >= 2
        half = grp % 2
        key = (la, half)
        if key not in wcache:
            wq, wk, wv = wsel[la]
            sl = slice(half * 4, half * 4 + 4)
            wcache[key] = (_prep_w(wq[sl]), _prep_w(wk[sl]), _prep_w(wv[sl]))
        pwq, pwk, pwv = wcache[key]
        in_maps.append(
            {
                "xt": xts[(b, la)],
                "wq": pwq,
                "wk": pwk,
                "wv": pwv,
                "wo": wo_maps[la],
                "tri": tri,
                "idn": idn,
            }
        )
    return in_maps


def assemble(results, B=2):
    out = np.empty((B, N, D), np.float32)
    for c in range(8):
        b, grp = divmod(c, 4)
        part = np.asarray(results[c]["out"])  # (512, 1024)
        if grp >= 2:  # lookahead: un-reverse rows within each head block
            part = part.reshape(HPC, 128, D)[:, ::-1].reshape(512, D)
        out[b, grp * 512 : (grp + 1) * 512] = part
    return out


def kernel(x, Wq_lb, Wk_lb, Wv_lb, Wq_la, Wk_la, Wv_la, Wo):
    nc = _get_nc()
    in_maps = make_in_maps(x, Wq_lb, Wk_lb, Wv_lb, Wq_la, Wk_la, Wv_la, Wo)
    res = run_bass_kernel_spmd(nc, in_maps, list(range(8)))
    B, T, F_, D_ = x.shape
    return assemble(res.results, B).reshape(B, T, F_, D_)
